# revision 1
# baseline (speedup 1.0000x reference)
"""Attention_UP_decoder — Bass/Tile kernel for 8 trn2 NeuronCores.

Math (per batch element b, all conv1x1 so spatial is a flat free dim):
  xh = relu(s_h * (w_in_h @ x_h) + b_h)                  [512, 2304]
  xl = relu(s_l * (w_in_l @ x_l) + b_l)                  [256, 9216]
  xld = bilinear_down(xl)                                [256, 2304]
  md  = [xld; xh]                                        [768, 2304]
  E^T = exp(md^T @ w_A^T)   (softmax max/bias cancel)    [2304, 512]
  r   = colsum(E^T);  Bt^T = md^T @ w_B^T                [2304, 768]
  cw  = Bt^T.T @ E^T  (+ b_B correction via u1)          [768, 512]
  pool = rowmean(Bt) + b_B                               [768, 1]
  X2^T = (cw^T @ w_out1^T) * (1/r) + u1                  [512, 256]
  P^T  = w_W^T @ X2^T + w_out2^T                         [768, 256]
  PG1^T = w_G1^T @ P^T ; PG2^T = w_G2^T @ P^T
  out = PG1^T.T @ xl[own half] + up(PG2^T.T @ xh) + bias [256, 48, 96]

Everything after softmax/codeword is linear, so conv_G/conv_W/f/concat fold
into the small P-chain (exact algebra, verified vs reference ~6e-3 rms bf16).

Sharding: core k = (batch k//2, spatial half k%2). The half-asymmetry is
expressed purely in per-core DATA (cyclically row-rolled x inputs and
per-core interpolation weight maps) so all 8 cores run one SPMD program.
"""

import sys
import os

sys.path.insert(0, "/opt/trn_rl_repo")

import numpy as np

CH, CL, C, NCW = 512, 256, 768, 512
SD, SL = 48 * 48, 96 * 96
HALF = SL // 2
EPS = 1e-5
R_H = (-1, 23)  # xh row-roll per half (see module docstring)

_CACHE = {}


def _ceil_div(a, b):
    return (a + b - 1) // b


def _nchunks(total, step=512):
    out = []
    o = 0
    while o < total:
        out.append((o, min(step, total - o)))
        o += step
    return out


def _build():
    import concourse.bass as bass
    import concourse.mybir as mybir
    from concourse import tile
    from concourse import bacc

    dt = mybir.dt
    BF, F32 = dt.bfloat16, dt.float32
    AF = mybir.ActivationFunctionType
    from concourse.alu_op_type import AluOpType as OP

    nc = bacc.Bacc("TRN2", target_bir_lowering=False, debug=False, num_devices=8)

    D = {}

    def din(name, shape, dtp=BF):
        D[name] = nc.dram_tensor(name, list(shape), dtp, kind="ExternalInput").ap()

    din("xh_in", (CH, SD))
    din("xl_in", (CL, SL))
    din("w_in_hT", (CH, CH))
    din("w_in_lT", (CL, CL))
    din("w_AT", (C, NCW))
    din("w_BT", (C, C))
    din("w_out1T", (C, CL))
    din("w_out2T", (C, CL))
    din("w_W", (NCW, C))
    din("w_WT", (C, NCW))
    din("w_G", (C, C))
    din("s_h", (CH, 1), F32)
    din("bc_h", (CH, 1), F32)
    din("s_l", (CL, 1), F32)
    din("bc_l", (CL, 1), F32)
    din("b_B_f", (C, 1), F32)
    din("b_B_b", (C, 1))
    din("b_G_b", (C, 1))
    din("b_W_f", (NCW, 1), F32)
    din("b_out_f", (CL, 1), F32)
    din("mDc", (128, 48))
    din("mDr", (128, 48))
    din("mUce", (128, 47))
    din("mUco", (128, 47))
    din("mRe", (128, 24))
    din("mRo", (128, 24))
    din("ones_c", (128, 1))
    din("ones_r", (1, 128))
    out_d = nc.dram_tensor("out", [CL, HALF], F32, kind="ExternalOutput").ap()
    DBG = {}
    if os.environ.get("KDBG"):
        for nm, shp, dtp in [("d_xh", (CH, SD), BF), ("d_xld", (CL, SD), BF),
                             ("d_ET", (2304, NCW), BF), ("d_BtT", (2304, C), BF),
                             ("d_cw", (C, NCW), BF), ("d_rinv", (128, 4), F32),
                             ("d_X2T", (NCW, CL), BF), ("d_PT", (C, CL), BF),
                             ("d_Yh", (CL, SD), BF), ("d_th", (CL, HALF), BF),
                             ("d_pool", (C, 1), BF), ("d_biasc", (CL, 1), F32)]:
            DBG[nm] = nc.dram_tensor(nm, list(shp), dtp, kind="ExternalOutput").ap()

    with tile.TileContext(nc) as tc:
        _emit(tc, nc, D, out_d, BF, F32, AF, OP, bass, tile, DBG)
    nc.compile()
    return nc


def _emit(tc, nc, D, out_d, BF, F32, AF, OP, bass, tile, DBG=None):
    DBG = DBG or {}
    _dma_engs = [nc.sync, nc.scalar]
    _dma_i = [0]

    def dma(dst, src):
        e = _dma_engs[_dma_i[0] % len(_dma_engs)]
        _dma_i[0] += 1
        e.dma_start(dst, src)

    def dump(name, tiles, rows=128):
        if name in DBG:
            for i, t in enumerate(tiles):
                nc.sync.dma_start(DBG[name][i * rows : (i + 1) * rows, :], t[:, :])

    from contextlib import ExitStack

    es = ExitStack()
    P0 = es.enter_context(tc.tile_pool(name="pers", bufs=1))
    PSW = es.enter_context(tc.tile_pool(name="psw", bufs=4, space="PSUM"))
    PSA = es.enter_context(tc.tile_pool(name="psa", bufs=1, space="PSUM"))

    def load_rows(name, R, Cw, dtp=BF, tagp=None):
        tagp = tagp or name
        tiles = []
        for k in range(_ceil_div(R, 128)):
            r = min(128, R - k * 128)
            t = P0.tile([r, Cw], dtp, tag=f"{tagp}{k}", name=f"{tagp}{k}")
            dma(t[:, :], D[name][k * 128 : k * 128 + r, :])
            tiles.append(t)
        return tiles

    # ---- stem-l critical loads first ----------------------------------------
    w_in_lT = load_rows("w_in_lT", CL, CL)
    s_l = load_rows("s_l", CL, 1, F32)
    bc_l = load_rows("bc_l", CL, 1, F32)

    def bcast_map(name, w):
        full = P0.tile([128, w], BF, tag=f"{name}_b", name=f"{name}_b")
        dma(full[:, :], D[name][:, :])
        return full

    # persistent activations
    xh = [P0.tile([128, SD], BF, tag=f"xh{m}", name=f"xh{m}") for m in range(4)]
    xl = [P0.tile([128, SL], BF, tag=f"xl{m}", name=f"xl{m}") for m in range(2)]

    def mm_chain(psum, lhsT_sl, rhs_sl, first, last):
        nc.tensor.matmul(psum, lhsT_sl, rhs_sl, start=first, stop=last,
                         skip_group_check=True)

    # ---- stems ---------------------------------------------------------------
    with tc.tile_pool(name="sin", bufs=8) as SIN:
        for (no, nw) in _nchunks(SL):
            chunks = []
            for k in range(2):
                t = SIN.tile([128, nw], BF, tag="xin", name="xin")
                dma(t[:, :],
                    D["xl_in"][k * 128 : (k + 1) * 128, no : no + nw])
                chunks.append(t)
            for m in range(2):
                ps = PSW.tile([128, nw], F32, tag="ps", name="ps")
                for k in range(2):
                    mm_chain(ps[:, :], w_in_lT[k][:, m * 128 : (m + 1) * 128],
                             chunks[k][:, :], k == 0, k == 1)
                nc.scalar.activation(xl[m][:, no : no + nw], ps[:, :], AF.Relu,
                                     bias=bc_l[m][:, :], scale=s_l[m][:, :])
        w_in_hT = load_rows("w_in_hT", CH, CH)
        s_h = load_rows("s_h", CH, 1, F32)
        bc_h = load_rows("bc_h", CH, 1, F32)
        for (no, nw) in _nchunks(SD):
            chunks = []
            for k in range(4):
                t = SIN.tile([128, nw], BF, tag="xin", name="xin")
                dma(t[:, :],
                    D["xh_in"][k * 128 : (k + 1) * 128, no : no + nw])
                chunks.append(t)
            for m in range(4):
                ps = PSW.tile([128, nw], F32, tag="ps", name="ps")
                for k in range(4):
                    mm_chain(ps[:, :], w_in_hT[k][:, m * 128 : (m + 1) * 128],
                             chunks[k][:, :], k == 0, k == 3)
                nc.scalar.activation(xh[m][:, no : no + nw], ps[:, :], AF.Relu,
                                     bias=bc_h[m][:, :], scale=s_h[m][:, :])

    # maps + misc constants (needed from the downsample onward)
    mDc = bcast_map("mDc", 48)
    mDr = bcast_map("mDr", 48)
    mUce = bcast_map("mUce", 47)
    mUco = bcast_map("mUco", 47)
    mRe = bcast_map("mRe", 24)
    mRo = bcast_map("mRo", 24)
    ones = load_rows("ones_c", 128, 1)[0]
    ones_r = P0.tile([1, 128], BF, tag="ones_r", name="ones_r")
    dma(ones_r[:, :], D["ones_r"][:, :])
    b_B_f = load_rows("b_B_f", C, 1, F32)
    b_B_b = load_rows("b_B_b", C, 1)
    b_G_b = load_rows("b_G_b", C, 1)
    b_W_f = load_rows("b_W_f", NCW, 1, F32)
    b_out_f = load_rows("b_out_f", CL, 1, F32)

    # late weight loads (DMA overlaps stem/deep compute)
    w_AT = load_rows("w_AT", C, NCW)
    w_BT = load_rows("w_BT", C, C)
    w_out1T = load_rows("w_out1T", C, CL)
    w_out2T = load_rows("w_out2T", C, CL)
    w_W = load_rows("w_W", NCW, C)
    w_WT = load_rows("w_WT", C, NCW)
    w_G = load_rows("w_G", C, C)

    # ---- bilinear downsample xl -> xld (stride-2 decomposition) -------------
    xld = [P0.tile([128, SD], BF, tag=f"xld{m}", name=f"xld{m}") for m in range(2)]
    from concourse.bass import broadcast_tensor_aps

    def bmul(eng, out_ap, in_ap, map_ap):
        a, b = broadcast_tensor_aps(in_ap, map_ap)
        eng.tensor_tensor(out_ap, a, b, OP.mult)

    with tc.tile_pool(name="sdown", bufs=1) as SD_P:
        for m in range(2):
            r3 = xl[m][:, :].rearrange("p (h w) -> p h w", w=96)
            # row stage first (contiguous inner dim -> DVE fast path)
            E2, O2 = r3[:, 0:96:2, :], r3[:, 1:96:2, :]
            drow = SD_P.tile([128, 48 * 96], BF, tag=f"drow{m}", name=f"drow{m}")
            d3 = drow[:, :].rearrange("p (h w) -> p h w", w=96)
            tmp = SD_P.tile([128, 48 * 96], BF, tag=f"dtmp{m}", name=f"dtmp{m}")
            t3 = tmp[:, :].rearrange("p (h w) -> p h w", w=96)
            reng = nc.vector if m == 0 else nc.gpsimd
            reng.tensor_tensor(t3[:, :, :], O2, E2, OP.subtract)
            bmul(reng, t3[:, :, :], t3[:, :, :],
                 mDr[:, :].rearrange("p (h a) -> p h a", a=1))
            reng.tensor_tensor(d3[:, :, :], t3[:, :, :], E2, OP.add)
            # col stage (DVE)
            E, O = d3[:, :, 0:96:2], d3[:, :, 1:96:2]
            x3 = xld[m][:, :].rearrange("p (h w) -> p h w", w=48)
            g3 = t3[:, :, 0:48]
            nc.vector.tensor_tensor(g3, O, E, OP.subtract)
            bmul(nc.vector, g3, g3, mDc[:, :].rearrange("p (a w) -> p a w", a=1))
            nc.vector.tensor_tensor(x3[:, :, :], g3, E, OP.add)

    dump("d_xh", xh)
    dump("d_xld", xld)
    md = xld + xh  # 6 k-tiles of [128, 2304]

    # ---- deep branch: E^T, Bt^T, r, pool, cw --------------------------------
    with tc.tile_pool(name="sdeep", bufs=1) as SDP:
        ET = [SDP.tile([128, NCW], BF, tag=f"ET{m}", name=f"ET{m}") for m in range(18)]
        BtT = [SDP.tile([128, C], BF, tag=f"BtT{m}", name=f"BtT{m}") for m in range(18)]
        r_ps = PSA.tile([128, 4 * 18], F32, tag="r_ps", name="r_ps")
        pool_ps = PSA.tile([128, 6 * 18], F32, tag="pool_ps", name="pool_ps")
        KORD = [2, 3, 4, 5, 0, 1]  # xh tiles first: PE runs while downsample finishes
        for m in range(18):
            ps = PSW.tile([128, NCW], F32, tag="ps", name="ps")
            for ki, k in enumerate(KORD):
                mm_chain(ps[:, :], md[k][:, m * 128 : (m + 1) * 128], w_AT[k][:, :],
                         ki == 0, ki == 5)
            nc.scalar.activation(ET[m][:, :], ps[:, :], AF.Exp)
            for c4 in range(4):
                mm_chain(r_ps[:, c4 * 18 + m : c4 * 18 + m + 1],
                         ET[m][:, c4 * 128 : (c4 + 1) * 128], ones[:, :],
                         True, True)
            for (no, nw) in _nchunks(C, 384):
                psb = PSW.tile([128, 384], F32, tag="ps", name="ps")
                for ki, k in enumerate(KORD):
                    mm_chain(psb[:, 0:nw], md[k][:, m * 128 : (m + 1) * 128],
                             w_BT[k][:, no : no + nw], ki == 0, ki == 5)
                nc.scalar.copy(BtT[m][:, no : no + nw], psb[:, 0:nw])
            for c6 in range(6):
                mm_chain(pool_ps[:, c6 * 18 + m : c6 * 18 + m + 1],
                         BtT[m][:, c6 * 128 : (c6 + 1) * 128], ones[:, :],
                         True, True)

        cw = [P0.tile([128, NCW], BF, tag=f"cw{m}", name=f"cw{m}") for m in range(6)]
        for m in range(6):
            ps = PSW.tile([128, NCW], F32, tag="ps", name="ps")
            for k in range(18):
                mm_chain(ps[:, :], BtT[k][:, m * 128 : (m + 1) * 128], ET[k][:, :],
                         k == 0, k == 17)
            nc.scalar.copy(cw[m][:, :], ps[:, :])

        dump("d_ET", ET)
        dump("d_BtT", BtT)
        dump("d_cw", cw)
        import concourse.mybir as _mybir
        AX = _mybir.AxisListType
        rsum = P0.tile([128, 4], F32, tag="rsum", name="rsum")
        nc.vector.tensor_reduce(rsum[:, :],
                                r_ps[:, :].rearrange("p (c m) -> p c m", m=18),
                                AX.X, OP.add)
        rinv = P0.tile([128, 4], F32, tag="rinv", name="rinv")
        nc.vector.reciprocal(rinv[:, :], rsum[:, :])
        psum_r = P0.tile([128, 6], F32, tag="psum_r", name="psum_r")
        nc.vector.tensor_reduce(psum_r[:, :],
                                pool_ps[:, :].rearrange("p (c m) -> p c m", m=18),
                                AX.X, OP.add)
        pool_c = [P0.tile([128, 1], BF, tag=f"pool{m}", name=f"pool{m}") for m in range(6)]
        for m in range(6):
            nc.scalar.activation(pool_c[m][:, :], psum_r[:, m : m + 1], AF.Identity,
                                 bias=b_B_f[m][:, :], scale=1.0 / SD)

        dump("d_rinv", [rinv])
        dump("d_pool", pool_c, rows=1) if False else None
    if "d_pool" in DBG:
        for i, t in enumerate(pool_c):
            nc.sync.dma_start(DBG["d_pool"][i * 128 : (i + 1) * 128, :], t[:, :])

    # ---- small P-chain -------------------------------------------------------
    u1ps = PSW.tile([1, CL], F32, tag="ps", name="ps")
    for k in range(6):
        mm_chain(u1ps[:, :], b_B_b[k][:, :], w_out1T[k][:, :], k == 0, k == 5)
    u1row = P0.tile([1, CL], BF, tag="u1row", name="u1row")
    nc.scalar.copy(u1row[:, :], u1ps[:, :])
    u1bp = PSW.tile([128, CL], F32, tag="ps", name="ps")
    mm_chain(u1bp[:, :], ones_r[:, :], u1row[:, :], True, True)
    u1b = P0.tile([128, CL], F32, tag="u1b", name="u1b")
    nc.scalar.copy(u1b[:, :], u1bp[:, :])

    X2T = [P0.tile([128, CL], BF, tag=f"X2T{m}", name=f"X2T{m}") for m in range(4)]
    for m in range(4):
        ps = PSW.tile([128, CL], F32, tag="ps", name="ps")
        for k in range(6):
            mm_chain(ps[:, :], cw[k][:, m * 128 : (m + 1) * 128], w_out1T[k][:, :],
                     k == 0, k == 5)
        nc.vector.scalar_tensor_tensor(X2T[m][:, :], ps[:, :], rinv[:, m : m + 1],
                                       u1b[:, :], OP.mult, OP.add)

    PT = [P0.tile([128, CL], BF, tag=f"PT{m}", name=f"PT{m}") for m in range(6)]
    for m in range(6):
        ps = PSW.tile([128, CL], F32, tag="ps", name="ps")
        for k in range(4):
            mm_chain(ps[:, :], w_W[k][:, m * 128 : (m + 1) * 128], X2T[k][:, :],
                     k == 0, k == 3)
        nc.vector.tensor_tensor(PT[m][:, :], ps[:, :], w_out2T[m][:, :], OP.add)

    dump("d_X2T", X2T)
    dump("d_PT", PT)
    PG1T = [P0.tile([128, CL], BF, tag=f"PG1T{m}", name=f"PG1T{m}") for m in range(2)]
    for m in range(2):
        ps = PSW.tile([128, CL], F32, tag="ps", name="ps")
        for k in range(6):
            mm_chain(ps[:, :], w_G[k][:, m * 128 : (m + 1) * 128], PT[k][:, :],
                     k == 0, k == 5)
        nc.scalar.copy(PG1T[m][:, :], ps[:, :])
    PG2T = [P0.tile([128, CL], BF, tag=f"PG2T{m}", name=f"PG2T{m}") for m in range(4)]
    for m in range(4):
        ps = PSW.tile([128, CL], F32, tag="ps", name="ps")
        for k in range(6):
            mm_chain(ps[:, :], w_G[k][:, CL + m * 128 : CL + (m + 1) * 128],
                     PT[k][:, :], k == 0, k == 5)
        nc.scalar.copy(PG2T[m][:, :], ps[:, :])

    tcol = [P0.tile([128, 1], BF, tag=f"tcol{m}", name=f"tcol{m}") for m in range(4)]
    for m in range(4):
        ps = PSW.tile([128, 1], F32, tag="ps", name="ps")
        for k in range(6):
            mm_chain(ps[:, :], w_WT[k][:, m * 128 : (m + 1) * 128], pool_c[k][:, :],
                     k == 0, k == 5)
        nc.scalar.activation(tcol[m][:, :], ps[:, :], AF.Identity, bias=b_W_f[m][:, :])

    biasc = [P0.tile([128, 1], F32, tag=f"biasc{m}", name=f"biasc{m}") for m in range(2)]
    for m in range(2):
        ps = PSW.tile([128, 1], F32, tag="ps", name="ps")
        for k in range(4):
            mm_chain(ps[:, :], X2T[k][:, m * 128 : (m + 1) * 128], tcol[k][:, :],
                     k == 0, False)
        for k in range(6):
            mm_chain(ps[:, :], PT[k][:, m * 128 : (m + 1) * 128], b_G_b[k][:, :],
                     False, k == 5)
        nc.scalar.activation(biasc[m][:, :], ps[:, :], AF.Identity,
                             bias=b_out_f[m][:, :])

    # ---- Yh + bilinear upsample (rolled windows; see docstring) -------------
    TAIL = es.enter_context(tc.tile_pool(name="tail", bufs=1))
    Yh = [TAIL.tile([128, SD], BF, tag=f"Yh{m}", name=f"Yh{m}") for m in range(2)]
    for m in range(2):
        for (no, nw) in _nchunks(SD):
            ps = PSW.tile([128, nw], F32, tag="ps", name="ps")
            for k in range(4):
                mm_chain(ps[:, :], PG2T[k][:, m * 128 : (m + 1) * 128],
                         xh[k][:, no : no + nw], k == 0, k == 3)
            nc.scalar.copy(Yh[m][:, no : no + nw], ps[:, :])

    th = [TAIL.tile([128, HALF], BF, tag=f"th{m}", name=f"th{m}") for m in range(2)]
    with tc.tile_pool(name="sup", bufs=1) as SUP:
        for m in range(2):
            Yr = Yh[m][:, :].rearrange("p (h w) -> p h w", w=48)
            rowup = SUP.tile([128, SD], BF, tag=f"ru{m}", name=f"ru{m}")
            ru = rowup[:, :].rearrange("p (h w) -> p h w", w=48)
            tmp = SUP.tile([128, 24 * 48], BF, tag=f"ut{m}", name=f"ut{m}")
            t3 = tmp[:, :].rearrange("p (h w) -> p h w", w=48)
            # even local rows: W0=Yr[e], W1=Yr[e+1]; out = W1 + mRe*(W0-W1)
            nc.vector.tensor_tensor(t3[:, :, :], Yr[:, 0:24, :], Yr[:, 1:25, :],
                                    OP.subtract)
            bmul(nc.vector, t3[:, :, :], t3[:, :, :],
                 mRe[:, :].rearrange("p (h a) -> p h a", a=1))
            nc.vector.tensor_tensor(ru[:, 0:48:2, :], t3[:, :, :], Yr[:, 1:25, :],
                                    OP.add)
            # odd local rows: W0=Yr[e+1], W1=Yr[e+2]; out = W1 + mRo*(W0-W1)
            nc.vector.tensor_tensor(t3[:, :, :], Yr[:, 1:25, :], Yr[:, 2:26, :],
                                    OP.subtract)
            bmul(nc.vector, t3[:, :, :], t3[:, :, :],
                 mRo[:, :].rearrange("p (h a) -> p h a", a=1))
            nc.vector.tensor_tensor(ru[:, 1:48:2, :], t3[:, :, :], Yr[:, 2:26, :],
                                    OP.add)
            # col upsample 48 -> 96 (m=0 on DVE, m=1 on GpSimd)
            eng = nc.vector if m == 0 else nc.gpsimd
            o3 = th[m][:, :].rearrange("p (h w) -> p h w", w=96)
            W0, W1 = ru[:, :, 0:47], ru[:, :, 1:48]
            ctmp = SUP.tile([128, 48 * 47], BF, tag=f"ct{m}", name=f"ct{m}")
            c3 = ctmp[:, :].rearrange("p (h w) -> p h w", w=47)
            # even cols 2..94: in[t] + mUce*(in[t-1]-in[t]), t=1..47
            eng.tensor_tensor(c3[:, :, :], W0, W1, OP.subtract)
            bmul(eng, c3[:, :, :], c3[:, :, :],
                 mUce[:, :].rearrange("p (a w) -> p a w", a=1))
            eng.tensor_tensor(o3[:, :, 2:96:2], c3[:, :, :], W1, OP.add)
            eng.tensor_copy(o3[:, :, 0:1], ru[:, :, 0:1])
            # odd cols 1..93: in[t] + mUco*(in[t+1]-in[t]), t=0..46
            eng.tensor_tensor(c3[:, :, :], W1, W0, OP.subtract)
            bmul(eng, c3[:, :, :], c3[:, :, :],
                 mUco[:, :].rearrange("p (a w) -> p a w", a=1))
            eng.tensor_tensor(o3[:, :, 1:95:2], c3[:, :, :], W0, OP.add)
            eng.tensor_copy(o3[:, :, 95:96], ru[:, :, 47:48])

    dump("d_Yh", Yh)
    dump("d_th", th)
    dump("d_biasc", biasc)
    # ---- tl + combine + store ----------------------------------------------
    with tc.tile_pool(name="sout", bufs=3) as SOUT:
        for m in range(2):
            for (no, nw) in _nchunks(HALF):
                ps = PSW.tile([128, nw], F32, tag="ps", name="ps")
                for k in range(2):
                    mm_chain(ps[:, :], PG1T[k][:, m * 128 : (m + 1) * 128],
                             xl[k][:, 192 + no : 192 + no + nw], k == 0, k == 1)
                ot = SOUT.tile([128, 512], F32, tag="ot", name="ot")
                nc.scalar.activation(ot[:, 0:nw], ps[:, :], AF.Identity,
                                     bias=biasc[m][:, :])
                nc.vector.tensor_tensor(ot[:, 0:nw], ot[:, 0:nw],
                                        th[m][:, no : no + nw], OP.add)
                dma(out_d[m * 128 : (m + 1) * 128, no : no + nw], ot[:, 0:nw])
    es.close()


def _prep_inputs(x_h, x_l, w):
    import ml_dtypes

    bf16 = ml_dtypes.bfloat16
    f32 = np.float32

    def b(a):
        return np.ascontiguousarray(a, dtype=f32).astype(bf16)

    def f(a):
        return np.ascontiguousarray(a, dtype=f32)

    shared = {
        "w_in_hT": b(w["w_in_h"].T),
        "w_in_lT": b(w["w_in_l"].T),
        "w_AT": b(w["w_A"].T),
        "w_BT": b(w["w_B"].T),
        "w_out1T": b(w["w_out"][:, :C].T),
        "w_out2T": b(w["w_out"][:, C:].T),
        "w_W": b(w["w_W"]),
        "w_WT": b(w["w_W"].T),
        "w_G": b(w["w_G"]),
        "s_h": f(w["g_in_h"] / np.sqrt(1.0 + EPS)).reshape(CH, 1),
        "bc_h": f(w["b_in_h"]).reshape(CH, 1),
        "s_l": f(w["g_in_l"] / np.sqrt(1.0 + EPS)).reshape(CL, 1),
        "bc_l": f(w["b_in_l"]).reshape(CL, 1),
        "b_B_f": f(w["b_B"]).reshape(C, 1),
        "b_B_b": b(w["b_B"]).reshape(C, 1),
        "b_G_b": b(w["b_G"]).reshape(C, 1),
        "b_W_f": f(w["b_W"]).reshape(NCW, 1),
        "b_out_f": f(w["b_out"]).reshape(CL, 1),
        "mDc": np.tile(b(np.arange(48) / 47.0).reshape(1, 48), (128, 1)),
        "mUce": np.tile(b(np.arange(1, 48) / 95.0).reshape(1, 47), (128, 1)),
        "mUco": np.tile(b((47.0 - np.arange(47)) / 95.0).reshape(1, 47), (128, 1)),
        "ones_c": b(np.ones((128, 1))),
        "ones_r": b(np.ones((1, 128))),
    }
    in_maps = []
    for core in range(8):
        bidx, j = core // 2, core % 2
        rh = R_H[j]
        m = dict(shared)
        m["xh_in"] = b(np.roll(x_h[bidx], -rh, axis=1).reshape(CH, SD))
        m["xl_in"] = b(np.roll(x_l[bidx], -2 * rh, axis=1).reshape(CL, SL))
        m["mDr"] = np.tile(b(((np.arange(48) + rh) % 48) / 47.0).reshape(1, 48), (128, 1))
        m["mRe"] = np.tile(b((np.arange(24) + 24 * j) / 95.0).reshape(1, 24), (128, 1))
        m["mRo"] = np.tile(b((48 + np.arange(24) + 24 * j) / 95.0).reshape(1, 24), (128, 1))
        in_maps.append(m)
    return in_maps


def kernel(x_h, x_l, **w):
    import time

    if "nc" not in _CACHE:
        _CACHE["nc"] = _build()
    nc = _CACHE["nc"]

    x_h = np.asarray(x_h, np.float32)
    x_l = np.asarray(x_l, np.float32)
    in_maps = _prep_inputs(x_h, x_l, w)

    from concourse.bass_utils import run_bass_kernel_spmd

    t0 = time.perf_counter()
    res = run_bass_kernel_spmd(nc, in_maps, core_ids=list(range(8)),
                               trace=bool(int(os.environ.get("KTRACE", "0"))))
    t1 = time.perf_counter()
    kernel._last_wall_ns = int((t1 - t0) * 1e9)
    kernel._last_exec_ns = res.exec_time_ns
    kernel._last_res = res

    out = np.zeros((4, CL, 96, 96), np.float32)
    for core in range(8):
        bidx, j = core // 2, core % 2
        o = np.asarray(res.results[core]["out"], np.float32).reshape(CL, 48, 96)
        out[bidx, :, 48 * j : 48 * (j + 1), :] = o
    return out

